# revision 37
# baseline (speedup 1.0000x reference)
"""Paged-attention decode kernel for 8 TRN2 NeuronCores (SPMD, data-parallel over sequences).

Problem: nn_Attention_15659450761267 (sparse_attention).
  S=64 seqs, H=32 query heads, HKV=8 kv heads (GQA g=4), D=128, BS=16,
  MAX_BLOCKS=128, T=2048, f32 caches [8192,16,8,128].

Sharding: core c owns sequences [8c, 8c+8). block_tables is arange
(spec fill), so sequence s's cache lives in blocks [128s, 128(s+1)) ->
its K/V cache is a contiguous [2048, 1024] f32 slab. Each core reads
only its own 8 slabs (134 MB) -> memory-roofline ~375us/core.

The reference scatters the new-token k/v into the cache at slot cl-1,
then attends over positions < cl. Equivalently (softmax is permutation
invariant): attend over cached positions t < cl-1 (masking out the
stale slot cl-1) plus the new (k, v) appended as an extra column.
No device-side scatter needed.

Pipeline (per sequence, per 128-position chunk):
  K chunk --PE transpose--> KT --DVE copy--> SBUF (f32r)
  ST[t, (h,g)] = KT_h.T @ qt_h          (8 small fp32r matmuls -> one PSUM tile)
  p~ = exp(ST + mask_col)               (ONE ACT op, PSUM->SBUF, f32r out;
                                         mask col = -1e30 where pos >= cl-1)
  PV  += p~.T @ V_chunk                 (fp32r matmuls, N=512)
  sums += p~.T @ ones                   (softmax denominators via ones-column)
Then the new token is appended as a K=1 matmul, and the epilogue does
out = PV * (1/sums) in one fused DVE pass before band-DMAs to DRAM.
No max-subtraction is needed: scores are O(+-8) after SCALE.
"""

import numpy as np

S = 64
H = 32
HKV = 8
G = H // HKV  # 4
D = 128
BS = 16
MAX_BLOCKS = 128
T = MAX_BLOCKS * BS  # 2048
SCALE = 0.08838834764831845
NCORES = 8
S_LOC = S // NCORES  # 8
NEG = -1.0e30
CHUNK = 128          # positions per chunk (one transpose / ST tile)
NCHUNK = T // CHUNK  # 16
BLK = 512            # positions per K-load block
NBLK = T // BLK      # 4
CPB = BLK // CHUNK   # 4

_nc_cache = {}


def _build_nc(chunk_counts=(NCHUNK,) * S_LOC, reps=1):
    import concourse.mybir as mybir
    import concourse.tile as tile
    from concourse import bacc
    from concourse.masks import make_identity

    f32 = mybir.dt.float32
    f32r = mybir.dt.float32r
    i32 = mybir.dt.int32
    Alu = mybir.AluOpType
    Act = mybir.ActivationFunctionType

    nc = bacc.Bacc("TRN2", target_bir_lowering=False, debug=False,
                   num_devices=NCORES)
    q_d = nc.dram_tensor("q", [S_LOC, H, D], f32, kind="ExternalInput")
    k_d = nc.dram_tensor("k", [S_LOC, HKV, D], f32, kind="ExternalInput")
    v_d = nc.dram_tensor("v", [S_LOC, HKV, D], f32r, kind="ExternalInput")
    kc_d = nc.dram_tensor("kc", [S_LOC, T, HKV * D], f32r, kind="ExternalInput")
    vc_d = nc.dram_tensor("vc", [S_LOC, T, HKV * D], f32r, kind="ExternalInput")
    cl_d = nc.dram_tensor("cl", [1, S_LOC], i32, kind="ExternalInput")
    out_d = nc.dram_tensor("out", [S_LOC, H, D], f32, kind="ExternalOutput")

    with tile.TileContext(nc) as tc:
        with (
            tc.tile_pool(name="const", bufs=1) as constp,
            tc.tile_pool(name="kchunk", bufs=3) as kpool,
            tc.tile_pool(name="vchunk", bufs=3) as vpool,
            tc.tile_pool(name="kt", bufs=3) as ktpool,
            tc.tile_pool(name="stexp", bufs=6) as stpool,
            tc.tile_pool(name="small", bufs=2) as smpool,
            tc.tile_pool(name="ps_ktp", bufs=2, space="PSUM") as ps_ktp,
            tc.tile_pool(name="ps_st", bufs=2, space="PSUM") as ps_st,
            tc.tile_pool(name="ps_pv", bufs=1, space="PSUM") as ps_pv,
            tc.tile_pool(name="ps_sums", bufs=1, space="PSUM") as ps_sums,
            tc.tile_pool(name="ps_small", bufs=1, space="PSUM") as ps_small,
        ):
            ident = constp.tile([128, 128], f32)
            make_identity(nc, ident[:])
            identr = constp.tile([128, 128], f32r)
            nc.vector.tensor_copy(identr[:], ident[:])
            onesf = constp.tile([128, G], f32)
            nc.vector.memset(onesf[:], 1.0)
            ones_r = constp.tile([128, G], f32r)
            nc.vector.tensor_copy(ones_r[:], onesf[:])

            # posCols[p, j] = j*128 + p  (position of partition p in chunk j)
            posc_i = constp.tile([CHUNK, NCHUNK], i32)
            nc.gpsimd.iota(posc_i[:], pattern=[[CHUNK, NCHUNK]], base=0,
                           channel_multiplier=1)
            posc = constp.tile([CHUNK, NCHUNK], f32)
            nc.vector.tensor_copy(posc[:], posc_i[:])

            # context_lens -> f32 (cl - 1), broadcast over 128 partitions
            cli = constp.tile([1, S_LOC], i32)
            nc.sync.dma_start(cli[:], cl_d[:])
            clf = constp.tile([1, S_LOC], f32)
            nc.vector.tensor_copy(clf[:], cli[:])
            nc.vector.tensor_scalar_add(clf[:], clf[:], -1.0)
            clb = constp.tile([CHUNK, S_LOC], f32)
            nc.gpsimd.partition_broadcast(clb[:], clf[:])

            for s in [ss for _ in range(reps) for ss in range(S_LOC)]:
                # ---- q / new-token k,v ----
                q_sb = smpool.tile([H, D], f32, tag="q")
                nc.sync.dma_start(q_sb[:], q_d[s])
                kn_sb = smpool.tile([HKV, D], f32, tag="kn")
                nc.sync.dma_start(kn_sb[:], k_d[s])
                vn_sb = smpool.tile([1, HKV * D], f32r, tag="vn")
                nc.sync.dma_start(
                    vn_sb[:], v_d.rearrange("s h d -> s (h d)")[s][None, :])

                # QT = q^T * SCALE  [D, H] (f32r)
                qt_ps = ps_small.tile([D, H], f32, tag="misc")
                nc.tensor.transpose(qt_ps[:], q_sb[:], ident[:H, :H])
                qt_sb = smpool.tile([D, H], f32r, tag="qt")
                nc.scalar.mul(qt_sb[:], qt_ps[:], SCALE)

                kc_v = kc_d[s].rearrange("(c p) d -> p c d", p=CHUNK)
                vc_v = vc_d[s].rearrange("(c p) d -> p c d", p=CHUNK)

                pv_ps = ps_pv.tile([H, HKV * D], f32, tag="pv")
                sums_ps = ps_sums.tile([H, G], f32, tag="sums")
                # ---- new token: p~_new row, appended as K=1 matmuls ----
                ktn_ps = ps_small.tile([D, HKV], f32, tag="misc")
                nc.tensor.transpose(ktn_ps[:], kn_sb[:], ident[:HKV, :HKV])
                ktn_sb = smpool.tile([D, HKV], f32r, tag="ktn_sb")
                nc.vector.tensor_copy(ktn_sb[:], ktn_ps[:])
                # p~_new as a [1, 32] row directly: per head a [1, 4] matmul
                ptn_ps = ps_small.tile([1, H], f32, tag="misc")
                for h in range(HKV):
                    nc.tensor.matmul(ptn_ps[:, G * h:G * (h + 1)],
                                     ktn_sb[:, h:h + 1],
                                     qt_sb[:, G * h:G * (h + 1)],
                                     start=True, stop=True)
                ptn_sb = smpool.tile([1, H], f32r, tag="ptn_sb")
                nc.scalar.activation(ptn_sb[:], ptn_ps[:], Act.Exp)
                ntstop = (chunk_counts[s] == 0)
                nc.tensor.matmul(pv_ps[:, :512], ptn_sb[:], vn_sb[:, :512],
                                 start=True, stop=ntstop)
                nc.tensor.matmul(pv_ps[:, 512:], ptn_sb[:], vn_sb[:, 512:],
                                 start=True, stop=ntstop)
                nc.tensor.matmul(sums_ps[:], ptn_sb[:], ones_r[:1, :],
                                 start=True, stop=ntstop)


                nch = chunk_counts[s]
                nblocks = (nch + CPB - 1) // CPB
                for b in range(nblocks):
                    cpb = min(CPB, nch - b * CPB)
                    k_sb = kpool.tile([CHUNK, CPB, HKV * D], f32r,
                                      tag="kchunk")
                    nc.sync.dma_start(
                        k_sb[:, :cpb], kc_v[:, b * CPB:b * CPB + cpb, :])
                    v_sb = vpool.tile([CHUNK, CPB, HKV * D], f32r,
                                      tag="vchunk")
                    nc.sync.dma_start(
                        v_sb[:, :cpb], vc_v[:, b * CPB:b * CPB + cpb, :])

                    # K^T for this block: per head, PE transposes into one
                    # PSUM bank, one wide DVE copy to SBUF (f32r).
                    kt = ktpool.tile([D, HKV, BLK], f32r, tag="kt")
                    for h in range(HKV):
                        ktp = ps_ktp.tile([D, BLK], f32r, tag="ktp")
                        for c2 in range(cpb):
                            nc.tensor.transpose(
                                ktp[:, c2 * CHUNK:(c2 + 1) * CHUNK],
                                k_sb[:, c2, h * D:(h + 1) * D], identr[:])
                        nc.vector.tensor_copy(kt[:, h, :cpb * CHUNK],
                                              ktp[:, :cpb * CHUNK])

                    for c2 in range(cpb):
                        c = b * CPB + c2
                        # ST[t, (h,g)] = k_t . q_(h,g) * SCALE (transposed!)
                        st_ps = ps_st.tile([CHUNK, H], f32, tag="st")
                        for h in range(HKV):
                            nc.tensor.matmul(
                                st_ps[:, G * h:G * (h + 1)],
                                kt[:, h, c2 * CHUNK:(c2 + 1) * CHUNK],
                                qt_sb[:, G * h:G * (h + 1)],
                                start=True, stop=True)
                        # mask column: -1e30 where position >= cl-1
                        mc = smpool.tile([CHUNK, 1], f32, tag="mc")
                        nc.vector.tensor_scalar(
                            mc[:], posc[:, c:c + 1], clb[:, s:s + 1], NEG,
                            op0=Alu.is_ge, op1=Alu.mult)
                        # p~ = exp(ST + mask): one ACT op, PSUM -> SBUF f32r
                        st_exp = stpool.tile([CHUNK, H], f32r, tag="stexp")
                        nc.scalar.activation(st_exp[:], st_ps[:], Act.Exp,
                                             bias=mc[:, 0:1])
                        # PV and denominator accumulation
                        last = (c == nch - 1)
                        nc.tensor.matmul(pv_ps[:, :512], st_exp[:],
                                         v_sb[:, c2, :512],
                                         start=False, stop=last)
                        nc.tensor.matmul(pv_ps[:, 512:], st_exp[:],
                                         v_sb[:, c2, 512:],
                                         start=False, stop=last)
                        nc.tensor.matmul(sums_ps[:], st_exp[:], ones_r[:],
                                         start=False, stop=last)

                # ---- epilogue: out = PV / sums, band-DMA to DRAM ----
                sums_sb = smpool.tile([H, 1], f32, tag="sums_sb")
                nc.vector.tensor_copy(sums_sb[:], sums_ps[:, 0:1])
                rcp = smpool.tile([H, 1], f32, tag="rcp")
                nc.vector.reciprocal(rcp[:], sums_sb[:])
                pv_stage = smpool.tile([H, HKV * D], f32, tag="pvstage")
                nc.vector.tensor_scalar(pv_stage[:], pv_ps[:], rcp[:, 0:1],
                                        None, op0=Alu.mult)
                for h in range(HKV):
                    nc.scalar.dma_start(
                        out_d[s, G * h:G * (h + 1), :],
                        pv_stage[G * h:G * (h + 1), h * D:(h + 1) * D])

    nc.compile()
    return nc


def _get_nc(chunk_counts):
    key = tuple(chunk_counts)
    if key not in _nc_cache:
        _nc_cache[key] = _build_nc(chunk_counts=key)
    return _nc_cache[key]


def _plan(q, k, v, k_cache, v_cache, block_tables, context_lens,
          slot_mapping):
    """Sort sequences by context length, snake-deal to (core, slot), and
    compute per-slot static chunk counts (max over cores in each slot)."""
    q = np.ascontiguousarray(np.asarray(q, np.float32))
    k = np.ascontiguousarray(np.asarray(k, np.float32))
    v = np.ascontiguousarray(np.asarray(v, np.float32))
    kc = np.asarray(k_cache, np.float32)
    vc = np.asarray(v_cache, np.float32)
    bt = np.asarray(block_tables)
    cl = np.asarray(context_lens, np.int32)

    expect = np.arange(S * MAX_BLOCKS, dtype=np.int64).reshape(S, MAX_BLOCKS)
    if not np.array_equal(np.asarray(bt, np.int64), expect):
        # General fallback (never hit for the spec's arange tables): gather
        # each sequence's blocks into contiguous order on the host.
        kc = kc[np.asarray(bt, np.int64)].reshape(S, T, HKV * D)
        vc = vc[np.asarray(bt, np.int64)].reshape(S, T, HKV * D)
    else:
        kc = kc.reshape(S, T, HKV * D)
        vc = vc.reshape(S, T, HKV * D)

    # cached chunks needed for positions 0 .. cl-2
    need = np.ceil(np.maximum(cl - 1, 0) / CHUNK).astype(np.int64)
    order = np.argsort(-need, kind="stable")  # desc by need
    # snake deal: rank group j -> slot j; within group alternate direction
    assign = np.empty((NCORES, S_LOC), np.int64)
    for j in range(S_LOC):
        grp = order[j * NCORES:(j + 1) * NCORES]
        if j % 2 == 1:
            grp = grp[::-1]
        assign[:, j] = grp
    chunk_counts = tuple(int(need[assign[:, j]].max()) for j in range(S_LOC))

    in_maps = []
    for c in range(NCORES):
        idx = assign[c]
        in_maps.append({
            "q": np.ascontiguousarray(q[idx]),
            "k": np.ascontiguousarray(k[idx]),
            "v": np.ascontiguousarray(v[idx]),
            "kc": np.ascontiguousarray(kc[idx]),
            "vc": np.ascontiguousarray(vc[idx]),
            "cl": np.ascontiguousarray(cl[idx]).reshape(1, S_LOC),
        })
    return in_maps, assign, chunk_counts


def _prep_shards(q, k, v, k_cache, v_cache, block_tables, context_lens,
                 slot_mapping):
    in_maps, _, _ = _plan(q, k, v, k_cache, v_cache, block_tables,
                          context_lens, slot_mapping)
    return in_maps


def kernel(q, k, v, k_cache, v_cache, block_tables, context_lens,
           slot_mapping) -> np.ndarray:
    from concourse.bass_utils import run_bass_kernel_spmd

    in_maps, assign, chunk_counts = _plan(
        q, k, v, k_cache, v_cache, block_tables, context_lens, slot_mapping)
    nc = _get_nc(chunk_counts)
    res = run_bass_kernel_spmd(nc, in_maps, core_ids=list(range(NCORES)),
                               trace=False)
    out = np.empty((S, H, D), np.float32)
    for c in range(NCORES):
        out[assign[c]] = res.results[c]["out"]
    return np.ascontiguousarray(out)


# revision 40
# speedup vs baseline: 1.1204x; 1.1204x over previous
"""Paged-attention decode kernel for 8 TRN2 NeuronCores (SPMD, data-parallel over sequences).

Problem: nn_Attention_15659450761267 (sparse_attention).
  S=64 seqs, H=32 query heads, HKV=8 kv heads (GQA g=4), D=128, BS=16,
  MAX_BLOCKS=128, T=2048, f32 caches [8192,16,8,128].

Sharding: core c owns sequences [8c, 8c+8). block_tables is arange
(spec fill), so sequence s's cache lives in blocks [128s, 128(s+1)) ->
its K/V cache is a contiguous [2048, 1024] f32 slab. Each core reads
only its own 8 slabs (134 MB) -> memory-roofline ~375us/core.

The reference scatters the new-token k/v into the cache at slot cl-1,
then attends over positions < cl. Equivalently (softmax is permutation
invariant): attend over cached positions t < cl-1 (masking out the
stale slot cl-1) plus the new (k, v) appended as an extra column.
No device-side scatter needed.

Pipeline (per sequence, per 128-position chunk):
  K chunk --PE transpose--> KT --DVE copy--> SBUF (f32r)
  ST[t, (h,g)] = KT_h.T @ qt_h          (8 small fp32r matmuls -> one PSUM tile)
  p~ = exp(ST + mask_col)               (ONE ACT op, PSUM->SBUF, f32r out;
                                         mask col = -1e30 where pos >= cl-1)
  PV  += p~.T @ V_chunk                 (fp32r matmuls, N=512)
  sums += p~.T @ ones                   (softmax denominators via ones-column)
Then the new token is appended as a K=1 matmul, and the epilogue does
out = PV * (1/sums) in one fused DVE pass before band-DMAs to DRAM.
No max-subtraction is needed: scores are O(+-8) after SCALE.
"""

import numpy as np

S = 64
H = 32
HKV = 8
G = H // HKV  # 4
D = 128
BS = 16
MAX_BLOCKS = 128
T = MAX_BLOCKS * BS  # 2048
SCALE = 0.08838834764831845
NCORES = 8
S_LOC = S // NCORES  # 8
NEG = -1.0e30
CHUNK = 128          # positions per chunk (one transpose / ST tile)
NCHUNK = T // CHUNK  # 16
BLK = 512            # positions per K-load block
NBLK = T // BLK      # 4
CPB = BLK // CHUNK   # 4

_nc_cache = {}


def _build_nc(chunk_counts=(NCHUNK,) * S_LOC, reps=1, ktp_bufs=2, st_bufs=2, kt_bufs=2, kv_bufs=2):
    import concourse.mybir as mybir
    import concourse.tile as tile
    from concourse import bacc
    from concourse.masks import make_identity

    f32 = mybir.dt.float32
    f32r = mybir.dt.float32r
    i32 = mybir.dt.int32
    Alu = mybir.AluOpType
    Act = mybir.ActivationFunctionType

    nc = bacc.Bacc("TRN2", target_bir_lowering=False, debug=False,
                   num_devices=NCORES)
    q_d = nc.dram_tensor("q", [S_LOC, H, D], f32, kind="ExternalInput")
    k_d = nc.dram_tensor("k", [S_LOC, HKV, D], f32, kind="ExternalInput")
    v_d = nc.dram_tensor("v", [S_LOC, HKV, D], f32r, kind="ExternalInput")
    kc_d = nc.dram_tensor("kc", [S_LOC, T, HKV * D], f32r, kind="ExternalInput")
    vc_d = nc.dram_tensor("vc", [S_LOC, T, HKV * D], f32r, kind="ExternalInput")
    cl_d = nc.dram_tensor("cl", [1, S_LOC], i32, kind="ExternalInput")
    out_d = nc.dram_tensor("out", [S_LOC, H, D], f32, kind="ExternalOutput")

    with tile.TileContext(nc) as tc:
        with (
            tc.tile_pool(name="const", bufs=1) as constp,
            tc.tile_pool(name="kchunk", bufs=kv_bufs) as kpool,
            tc.tile_pool(name="vchunk", bufs=kv_bufs) as vpool,
            tc.tile_pool(name="kt", bufs=kt_bufs) as ktpool,
            tc.tile_pool(name="stexp", bufs=6) as stpool,
            tc.tile_pool(name="small", bufs=2) as smpool,
            tc.tile_pool(name="ps_ktp", bufs=ktp_bufs, space="PSUM") as ps_ktp,
            tc.tile_pool(name="ps_st", bufs=st_bufs, space="PSUM") as ps_st,
            tc.tile_pool(name="ps_pv", bufs=1, space="PSUM") as ps_pv,
            tc.tile_pool(name="ps_sums", bufs=1, space="PSUM") as ps_sums,
            tc.tile_pool(name="ps_small", bufs=1, space="PSUM") as ps_small,
        ):
            ident = constp.tile([128, 128], f32)
            make_identity(nc, ident[:])
            identr = constp.tile([128, 128], f32r)
            nc.vector.tensor_copy(identr[:], ident[:])
            onesf = constp.tile([128, G], f32)
            nc.vector.memset(onesf[:], 1.0)
            ones_r = constp.tile([128, G], f32r)
            nc.vector.tensor_copy(ones_r[:], onesf[:])

            # posCols[p, j] = j*128 + p  (position of partition p in chunk j)
            posc_i = constp.tile([CHUNK, NCHUNK], i32)
            nc.gpsimd.iota(posc_i[:], pattern=[[CHUNK, NCHUNK]], base=0,
                           channel_multiplier=1)
            posc = constp.tile([CHUNK, NCHUNK], f32)
            nc.vector.tensor_copy(posc[:], posc_i[:])

            # context_lens -> f32 (cl - 1), broadcast over 128 partitions
            cli = constp.tile([1, S_LOC], i32)
            nc.sync.dma_start(cli[:], cl_d[:])
            clf = constp.tile([1, S_LOC], f32)
            nc.vector.tensor_copy(clf[:], cli[:])
            nc.vector.tensor_scalar_add(clf[:], clf[:], -1.0)
            clb = constp.tile([CHUNK, S_LOC], f32)
            nc.gpsimd.partition_broadcast(clb[:], clf[:])

            for s in [ss for _ in range(reps) for ss in range(S_LOC)]:
                # ---- q / new-token k,v ----
                q_sb = smpool.tile([H, D], f32, tag="q")
                nc.sync.dma_start(q_sb[:], q_d[s])
                kn_sb = smpool.tile([HKV, D], f32, tag="kn")
                nc.sync.dma_start(kn_sb[:], k_d[s])
                vn_sb = smpool.tile([1, HKV * D], f32r, tag="vn")
                nc.sync.dma_start(
                    vn_sb[:], v_d.rearrange("s h d -> s (h d)")[s][None, :])

                # QT = q^T * SCALE  [D, H] (f32r)
                qt_ps = ps_small.tile([D, H], f32, tag="misc")
                nc.tensor.transpose(qt_ps[:], q_sb[:], ident[:H, :H])
                qt_sb = smpool.tile([D, H], f32r, tag="qt")
                nc.scalar.mul(qt_sb[:], qt_ps[:], SCALE)

                kc_v = kc_d[s].rearrange("(c p) d -> p c d", p=CHUNK)
                vc_v = vc_d[s].rearrange("(c p) d -> p c d", p=CHUNK)

                pv_ps = ps_pv.tile([H, HKV * D], f32, tag="pv")
                sums_ps = ps_sums.tile([H, G], f32, tag="sums")
                # ---- new token: p~_new row, appended as K=1 matmuls ----
                ktn_ps = ps_small.tile([D, HKV], f32, tag="misc")
                nc.tensor.transpose(ktn_ps[:], kn_sb[:], ident[:HKV, :HKV])
                ktn_sb = smpool.tile([D, HKV], f32r, tag="ktn_sb")
                nc.vector.tensor_copy(ktn_sb[:], ktn_ps[:])
                # p~_new as a [1, 32] row directly: per head a [1, 4] matmul
                ptn_ps = ps_small.tile([1, H], f32, tag="misc")
                for h in range(HKV):
                    nc.tensor.matmul(ptn_ps[:, G * h:G * (h + 1)],
                                     ktn_sb[:, h:h + 1],
                                     qt_sb[:, G * h:G * (h + 1)],
                                     start=True, stop=True)
                ptn_sb = smpool.tile([1, H], f32r, tag="ptn_sb")
                nc.scalar.activation(ptn_sb[:], ptn_ps[:], Act.Exp)
                ntstop = (chunk_counts[s] == 0)
                nc.tensor.matmul(pv_ps[:, :512], ptn_sb[:], vn_sb[:, :512],
                                 start=True, stop=ntstop)
                nc.tensor.matmul(pv_ps[:, 512:], ptn_sb[:], vn_sb[:, 512:],
                                 start=True, stop=ntstop)
                nc.tensor.matmul(sums_ps[:], ptn_sb[:], ones_r[:1, :],
                                 start=True, stop=ntstop)


                nch = chunk_counts[s]
                nblocks = (nch + CPB - 1) // CPB
                for b in range(nblocks):
                    cpb = min(CPB, nch - b * CPB)
                    k_sb = kpool.tile([CHUNK, CPB, HKV * D], f32r,
                                      tag="kchunk")
                    nc.sync.dma_start(
                        k_sb[:, :cpb], kc_v[:, b * CPB:b * CPB + cpb, :])
                    v_sb = vpool.tile([CHUNK, CPB, HKV * D], f32r,
                                      tag="vchunk")
                    nc.sync.dma_start(
                        v_sb[:, :cpb], vc_v[:, b * CPB:b * CPB + cpb, :])

                    # K^T for this block: per head, PE transposes into one
                    # PSUM bank, one wide DVE copy to SBUF (f32r).
                    kt = ktpool.tile([D, HKV, BLK], f32r, tag="kt")
                    for h in range(HKV):
                        ktp = ps_ktp.tile([D, BLK], f32r, tag="ktp")
                        for c2 in range(cpb):
                            nc.tensor.transpose(
                                ktp[:, c2 * CHUNK:(c2 + 1) * CHUNK],
                                k_sb[:, c2, h * D:(h + 1) * D], identr[:])
                        nc.vector.tensor_copy(kt[:, h, :cpb * CHUNK],
                                              ktp[:, :cpb * CHUNK])

                    for c2 in range(cpb):
                        c = b * CPB + c2
                        # ST[t, (h,g)] = k_t . q_(h,g) * SCALE (transposed!)
                        st_ps = ps_st.tile([CHUNK, H], f32, tag="st")
                        for h in range(HKV):
                            nc.tensor.matmul(
                                st_ps[:, G * h:G * (h + 1)],
                                kt[:, h, c2 * CHUNK:(c2 + 1) * CHUNK],
                                qt_sb[:, G * h:G * (h + 1)],
                                start=True, stop=True)
                        # mask column: -1e30 where position >= cl-1
                        mc = smpool.tile([CHUNK, 1], f32, tag="mc")
                        nc.vector.tensor_scalar(
                            mc[:], posc[:, c:c + 1], clb[:, s:s + 1], NEG,
                            op0=Alu.is_ge, op1=Alu.mult)
                        # p~ = exp(ST + mask): one ACT op, PSUM -> SBUF f32r
                        st_exp = stpool.tile([CHUNK, H], f32r, tag="stexp")
                        nc.scalar.activation(st_exp[:], st_ps[:], Act.Exp,
                                             bias=mc[:, 0:1])
                        # PV and denominator accumulation
                        last = (c == nch - 1)
                        nc.tensor.matmul(pv_ps[:, :512], st_exp[:],
                                         v_sb[:, c2, :512],
                                         start=False, stop=last)
                        nc.tensor.matmul(pv_ps[:, 512:], st_exp[:],
                                         v_sb[:, c2, 512:],
                                         start=False, stop=last)
                        nc.tensor.matmul(sums_ps[:], st_exp[:], ones_r[:],
                                         start=False, stop=last)

                # ---- epilogue: out = PV / sums, band-DMA to DRAM ----
                sums_sb = smpool.tile([H, 1], f32, tag="sums_sb")
                nc.vector.tensor_copy(sums_sb[:], sums_ps[:, 0:1])
                rcp = smpool.tile([H, 1], f32, tag="rcp")
                nc.vector.reciprocal(rcp[:], sums_sb[:])
                pv_stage = smpool.tile([H, HKV * D], f32, tag="pvstage")
                nc.vector.tensor_scalar(pv_stage[:], pv_ps[:], rcp[:, 0:1],
                                        None, op0=Alu.mult)
                for h in range(HKV):
                    nc.scalar.dma_start(
                        out_d[s, G * h:G * (h + 1), :],
                        pv_stage[G * h:G * (h + 1), h * D:(h + 1) * D])

    nc.compile()
    return nc


def _get_nc(chunk_counts):
    key = tuple(chunk_counts)
    if key not in _nc_cache:
        _nc_cache[key] = _build_nc(chunk_counts=key)
    return _nc_cache[key]


def _plan(q, k, v, k_cache, v_cache, block_tables, context_lens,
          slot_mapping):
    """Sort sequences by context length, snake-deal to (core, slot), and
    compute per-slot static chunk counts (max over cores in each slot)."""
    q = np.ascontiguousarray(np.asarray(q, np.float32))
    k = np.ascontiguousarray(np.asarray(k, np.float32))
    v = np.ascontiguousarray(np.asarray(v, np.float32))
    kc = np.asarray(k_cache, np.float32)
    vc = np.asarray(v_cache, np.float32)
    bt = np.asarray(block_tables)
    cl = np.asarray(context_lens, np.int32)

    expect = np.arange(S * MAX_BLOCKS, dtype=np.int64).reshape(S, MAX_BLOCKS)
    if not np.array_equal(np.asarray(bt, np.int64), expect):
        # General fallback (never hit for the spec's arange tables): gather
        # each sequence's blocks into contiguous order on the host.
        kc = kc[np.asarray(bt, np.int64)].reshape(S, T, HKV * D)
        vc = vc[np.asarray(bt, np.int64)].reshape(S, T, HKV * D)
    else:
        kc = kc.reshape(S, T, HKV * D)
        vc = vc.reshape(S, T, HKV * D)

    # cached chunks needed for positions 0 .. cl-2
    need = np.ceil(np.maximum(cl - 1, 0) / CHUNK).astype(np.int64)
    order = np.argsort(-need, kind="stable")  # desc by need
    # snake deal: rank group j -> slot j; within group alternate direction
    assign = np.empty((NCORES, S_LOC), np.int64)
    for j in range(S_LOC):
        grp = order[j * NCORES:(j + 1) * NCORES]
        if j % 2 == 1:
            grp = grp[::-1]
        assign[:, j] = grp
    chunk_counts = tuple(int(need[assign[:, j]].max()) for j in range(S_LOC))

    in_maps = []
    for c in range(NCORES):
        idx = assign[c]
        in_maps.append({
            "q": np.ascontiguousarray(q[idx]),
            "k": np.ascontiguousarray(k[idx]),
            "v": np.ascontiguousarray(v[idx]),
            "kc": np.ascontiguousarray(kc[idx]),
            "vc": np.ascontiguousarray(vc[idx]),
            "cl": np.ascontiguousarray(cl[idx]).reshape(1, S_LOC),
        })
    return in_maps, assign, chunk_counts


def _prep_shards(q, k, v, k_cache, v_cache, block_tables, context_lens,
                 slot_mapping):
    in_maps, _, _ = _plan(q, k, v, k_cache, v_cache, block_tables,
                          context_lens, slot_mapping)
    return in_maps


def kernel(q, k, v, k_cache, v_cache, block_tables, context_lens,
           slot_mapping) -> np.ndarray:
    from concourse.bass_utils import run_bass_kernel_spmd

    in_maps, assign, chunk_counts = _plan(
        q, k, v, k_cache, v_cache, block_tables, context_lens, slot_mapping)
    nc = _get_nc(chunk_counts)
    res = run_bass_kernel_spmd(nc, in_maps, core_ids=list(range(NCORES)),
                               trace=False)
    out = np.empty((S, H, D), np.float32)
    for c in range(NCORES):
        out[assign[c]] = res.results[c]["out"]
    return np.ascontiguousarray(out)
